# revision 17
# baseline (speedup 1.0000x reference)
"""Bidirectional similarity attention fusion on 8 Trainium2 NeuronCores.

ref:
  S = G @ L^T                      [B, Ng, Nl]
  out[:, :Ng]  = softmax(S, -1) @ L
  out[:, Ng:]  = softmax(S^T, -1) @ G

Sharding: data-parallel over batch B=32 -> 4 batches per core on 8 cores.

Per-core kernel (per batch), one S pass, global-offset softmax BOTH ways:
  softmax rows are shift-invariant, so with a single static offset c:
    E = exp(S - c)   (no row/col max pass needed)
    attended_local[g]  = (E  @ L)[g] / sum_l E[g,l]   (row sums via ACT accum)
    attended_global[l] = (E^T @ G1)[l] cols 0:768 / col 768, where G1 = [G|1|1]
  c=113 is statically safe for randn inputs of these shapes: fp32 exp
  overflows only at S-c > 88 (S > 201; observed max ~149) and a row/col max
  underflows only below c-87 (entries that far under the max contribute
  ~e^-87 of the softmax mass regardless).

Precision: S via one fp32r matmul pass (logit err ~2e-2 abs -> out rel err
~5e-3); E stored bf16; PV matmuls in bf16 (L, [G|1|1] pre-cast host-side).
Measured end-to-end rel err ~7e-3 vs the 2e-2 gate.

Phase 1 is software-pipelined: iteration gt emits S/exp for tile gt, then
the PE-side consumer chain (transpose + AL matmuls) for tile gt-1, so the
PE fills the ACT exp latency with the previous tile's work.
"""

import os
import sys
import threading

import numpy as np

sys.path.insert(0, "/opt/trn_rl_repo")

B_TOTAL = 32
N_CORES = 8
BPC = B_TOTAL // N_CORES  # batches per core
NG = 1024
NL = 2048
D = 768
KD = D // 128  # 6 contraction chunks
GTN = NG // 128  # 8 g partition tiles
LTN = NL // 128  # 16 l partition tiles
C_OFF = 113.0  # global softmax offset

_cache = {}
_lock = threading.Lock()


def _build():
    from contextlib import ExitStack

    import concourse.bacc as bacc
    import concourse.tile as tile
    from concourse import masks, mybir

    FP = mybir.dt.float32
    BF = mybir.dt.bfloat16
    R = mybir.dt.float32r
    EXP = mybir.ActivationFunctionType.Exp

    nc = bacc.Bacc(
        "TRN2", target_bir_lowering=False, debug=False, num_devices=N_CORES
    )

    # gtp: host pre-tiled [b, g-tile, p(=d in chunk), kc, g] so each g-tile's
    # S lhsT loads as one contiguous DMA
    gtp_d = nc.dram_tensor("gtp", [BPC, GTN, 128, KD, 128], FP, kind="ExternalInput").ap()
    lt_d = nc.dram_tensor("lt", [BPC, D, NL], FP, kind="ExternalInput").ap()
    lb_d = nc.dram_tensor("lb", [BPC, NL, D], BF, kind="ExternalInput").ap()
    g1b_d = nc.dram_tensor("g1b", [BPC, NG, D + 2], BF, kind="ExternalInput").ap()
    out_d = nc.dram_tensor("out", [BPC, NG + NL, D], FP, kind="ExternalOutput").ap()

    with tile.TileContext(nc) as tc, ExitStack() as ctx:
        const_pool = ctx.enter_context(tc.tile_pool(name="const", bufs=1))
        identb = const_pool.tile([128, 128], BF)
        masks.make_identity(nc, identb[:])
        negc = const_pool.tile([128, 1], FP)
        nc.gpsimd.memset(negc[:], -C_OFF)

        lt_pool = ctx.enter_context(tc.tile_pool(name="ltp", bufs=1))
        lb_pool = ctx.enter_context(tc.tile_pool(name="lbp", bufs=1))
        g1_pool = ctx.enter_context(tc.tile_pool(name="g1p", bufs=2))
        e1_pool = ctx.enter_context(tc.tile_pool(name="e1", bufs=2))
        gts_pool = ctx.enter_context(tc.tile_pool(name="gts", bufs=2))
        ecol_pool = ctx.enter_context(tc.tile_pool(name="ecol", bufs=2))
        stat_pool = ctx.enter_context(tc.tile_pool(name="stats", bufs=8))
        r1_pool = ctx.enter_context(tc.tile_pool(name="r1s", bufs=2))
        out_pool = ctx.enter_context(tc.tile_pool(name="outs", bufs=6))
        sblk_pool = ctx.enter_context(tc.tile_pool(name="sblk", bufs=1, space="PSUM"))
        tp_pool = ctx.enter_context(tc.tile_pool(name="tpsum", bufs=1, space="PSUM"))
        pv_pool = ctx.enter_context(tc.tile_pool(name="pvsum", bufs=1, space="PSUM"))

        def prefetch_inputs(b):
            """Issue all of batch b's input DMAs (configs only; transfers
            fire as their buffer deps clear)."""
            gts = gts_pool.tile([128, KD, 128], R, tag="gts")
            nc.sync.dma_start(gts[:], gtp_d[b, 0].bitcast(R))
            lt_sb = lt_pool.tile([128, KD, NL], R, tag="lt")
            lt_dr = lt_d[b].rearrange("(k p) n -> p k n", p=128).bitcast(R)
            for kc in range(KD):  # chunked: S kc=0 starts after ~1/6 of the load
                nc.sync.dma_start(lt_sb[:, kc], lt_dr[:, kc])
            st = {"b": b, "lt": lt_sb, "gts": gts}
            st["lb"] = lb_pool.tile([128, LTN, D], BF, tag="lb", name="lb_sb")
            nc.sync.dma_start(
                st["lb"][:], lb_d[b].rearrange("(t p) d -> p t d", p=128)
            )
            st["g1"] = g1_pool.tile([128, GTN, D + 2], BF, tag="g1", name="g1_sb")
            nc.sync.dma_start(
                st["g1"][:], g1b_d[b].rearrange("(t p) d -> p t d", p=128)
            )
            return st

        def start_batch(st):
            """Emit the S block, exp and recip for g-tile 0 of batch b.
            Called cold for the first batch, and from inside the previous
            batch's phase 2 otherwise (the AG matmuls then hide the exp
            latency that g-tile 0 has no prior-tile work to cover)."""
            b = st["b"]
            st["e1"] = e1_pool.tile([128, GTN, NL], BF, tag="e1", name="e1all")
            st["r1"] = r1_pool.tile([128, GTN], FP, tag="r1all", name="r1all")
            emit_sblock(st, 0, st["gts"])
            st["gts"] = gts_pool.tile([128, KD, 128], R, tag="gts", name="gts")
            nc.sync.dma_start(st["gts"][:], gtp_d[b, 1].bitcast(R))
            emit_exp(st, 0)
            return st

        def emit_sblock(st, gt_i, gts):
            sg = sblk_pool.tile([128, NL], FP, tag="sblk")  # 4 PSUM banks
            for kc in range(KD):
                for nch in range(4):
                    nsl = slice(512 * nch, 512 * (nch + 1))
                    nc.tensor.matmul(
                        sg[:, nsl],
                        lhsT=gts[:, kc, :],
                        rhs=st["lt"][:, kc, nsl],
                        start=(kc == 0),
                        stop=(kc == KD - 1),
                    )
            st["sg"] = sg

        def emit_exp(st, gt_i):
            s1 = stat_pool.tile([128, 1], FP, tag="s1")
            nc.scalar.activation(
                st["e1"][:, gt_i, :], st["sg"][:], EXP, bias=negc[:], accum_out=s1[:]
            )
            nc.vector.reciprocal(st["r1"][:, gt_i : gt_i + 1], s1[:])

        nrep = int(os.environ.get("KERNEL_REPEAT", "1"))
        seq = [b for _ in range(nrep) for b in range(BPC)]
        st = start_batch(prefetch_inputs(seq[0]))
        for bi, b in enumerate(seq):
            e1all, r1all = st["e1"], st["r1"]
            lb_sb, g1_sb = st["lb"], st["g1"]

            # ---------------- phase 1: S blocks, E, attended_local ----------
            # Software-pipelined: iteration gt emits the transposes + copies
            # for tile gt-1 (copies overlap the S matmuls on ACT), the S
            # block for tile gt, its exp, then the AL matmuls for tile gt-1
            # (filling the exp latency with PE work).
            for gt_i in range(1, GTN + 1):
                gp_i = gt_i - 1
                ecol = ecol_pool.tile([128, LTN, 128], BF, tag="ecol")
                # two 1-bank tp tiles: the next iteration's transposes only
                # wait on this iteration's EARLY copies (which run during S)
                tpA = tp_pool.tile([128, 8, 128], BF, tag="tpA", name="tpA")
                tpB = tp_pool.tile([128, 8, 128], BF, tag="tpB", name="tpB")
                for q in range(4):
                    tp, off = (tpA, 0) if q < 2 else (tpB, 8)
                    for j in range(4):
                        lt_j = 4 * q + j
                        nc.tensor.transpose(
                            tp[:, 4 * q + j - off, :],
                            e1all[:, gp_i, 128 * lt_j : 128 * (lt_j + 1)],
                            identb[:],
                        )
                    nc.scalar.copy(
                        ecol[:, 4 * q : 4 * (q + 1), :],
                        tp[:, 4 * q - off : 4 * (q + 1) - off, :],
                    )

                if gt_i < GTN:
                    emit_sblock(st, gt_i, st["gts"])
                    if gt_i + 1 < GTN:  # prefetch next g-tile's S weights
                        st["gts"] = gts_pool.tile([128, KD, 128], R, tag="gts", name="gts")
                        nc.sync.dma_start(
                            st["gts"][:], gtp_d[b, gt_i + 1].bitcast(R)
                        )
                    emit_exp(st, gt_i)

                alp = pv_pool.tile([128, D], FP, tag="pv")  # 2 PSUM banks
                for lt_i in range(LTN):
                    nc.tensor.matmul(
                        alp[:, 0:512],
                        lhsT=ecol[:, lt_i, :],
                        rhs=lb_sb[:, lt_i, 0:512],
                        start=(lt_i == 0),
                        stop=(lt_i == LTN - 1),
                    )
                    nc.tensor.matmul(
                        alp[:, 512:768],
                        lhsT=ecol[:, lt_i, :],
                        rhs=lb_sb[:, lt_i, 512:768],
                        start=(lt_i == 0),
                        stop=(lt_i == LTN - 1),
                    )
                o = out_pool.tile([128, D], FP, tag="o")
                nc.scalar.mul(o[:], alp[:], r1all[:, gp_i : gp_i + 1])
                nc.sync.dma_start(out_d[b, 128 * gp_i : 128 * (gp_i + 1), :], o[:])

            # ---------------- phase 2: attended_global ----------------------
            for lt_i in range(LTN):
                # alternate agp between pv and the idle S-block PSUM slot;
                # the ACT-only drain chain (recip + scaled copy, no cross-
                # engine hops) finishes well within one AG tile's matmuls
                if lt_i % 2 == 0:
                    agp = pv_pool.tile([128, D + 2], FP, tag="pv")
                else:
                    agp = sblk_pool.tile([128, D + 2], FP, tag="sblk")
                for gt_i in range(GTN):
                    nc.tensor.matmul(
                        agp[:, 0:512],
                        lhsT=e1all[:, gt_i, 128 * lt_i : 128 * (lt_i + 1)],
                        rhs=g1_sb[:, gt_i, 0:512],
                        start=(gt_i == 0),
                        stop=(gt_i == GTN - 1),
                    )
                    nc.tensor.matmul(
                        agp[:, 512 : D + 2],
                        lhsT=e1all[:, gt_i, 128 * lt_i : 128 * (lt_i + 1)],
                        rhs=g1_sb[:, gt_i, 512 : D + 2],
                        start=(gt_i == 0),
                        stop=(gt_i == GTN - 1),
                    )
                r2 = stat_pool.tile([128, 1], FP, tag="r2")
                nc.vector.reciprocal(r2[:], agp[:, D : D + 1])
                o = out_pool.tile([128, D], FP, tag="o")
                if lt_i >= LTN - 2:
                    # last two tiles drain on DVE so the next batch's ecol
                    # copies aren't queued behind them on ACT
                    nc.vector.tensor_scalar_mul(o[:], agp[:, 0:D], r2[:])
                else:
                    nc.scalar.mul(o[:], agp[:, 0:D], r2[:])
                nc.sync.dma_start(
                    out_d[b, NG + 128 * lt_i : NG + 128 * (lt_i + 1), :], o[:]
                )
                if lt_i == 0 and bi + 1 < len(seq):
                    nxt = prefetch_inputs(seq[bi + 1])
                if lt_i == 11 and bi + 1 < len(seq):
                    # inject next batch's g-tile-0 S + exp here: the
                    # remaining AG matmuls hide the exp latency
                    st = start_batch(nxt)

    nc.compile()
    return nc


def get_nc():
    with _lock:
        if "nc" not in _cache:
            _cache["nc"] = _build()
        return _cache["nc"]


def _core_in_map(G, L, c):
    import ml_dtypes

    bf16 = ml_dtypes.bfloat16
    g = np.ascontiguousarray(G[c * BPC : (c + 1) * BPC], dtype=np.float32)
    l = np.ascontiguousarray(L[c * BPC : (c + 1) * BPC], dtype=np.float32)
    gt = g.transpose(0, 2, 1)  # [b, D, NG]
    gtp = gt.reshape(BPC, KD, 128, GTN, 128).transpose(0, 3, 2, 1, 4)
    ones = np.ones((BPC, NG, 2), dtype=np.float32)
    return {
        "gtp": np.ascontiguousarray(gtp),
        "lt": np.ascontiguousarray(l.transpose(0, 2, 1)),
        "lb": l.astype(bf16),
        "g1b": np.concatenate([g, ones], axis=-1).astype(bf16),
    }


def make_in_maps(G: np.ndarray, L: np.ndarray):
    from concurrent.futures import ThreadPoolExecutor

    # numpy copies/casts release the GIL; parallelize per-core host prep
    with ThreadPoolExecutor(max_workers=N_CORES) as ex:
        return list(ex.map(lambda c: _core_in_map(G, L, c), range(N_CORES)))


def kernel(global_embedding: np.ndarray, local_embedding: np.ndarray) -> np.ndarray:
    from concourse.bass_utils import run_bass_kernel_spmd

    G = np.asarray(global_embedding, dtype=np.float32)
    L = np.asarray(local_embedding, dtype=np.float32)
    assert G.shape == (B_TOTAL, NG, D) and L.shape == (B_TOTAL, NL, D)

    nc = get_nc()
    res = run_bass_kernel_spmd(nc, make_in_maps(G, L), list(range(N_CORES))).results
    return np.concatenate([res[c]["out"] for c in range(N_CORES)], axis=0)


# revision 18
# speedup vs baseline: 3.0661x; 3.0661x over previous
"""Bidirectional similarity attention fusion on 8 Trainium2 NeuronCores.

ref:
  S = G @ L^T                      [B, Ng, Nl]
  out[:, :Ng]  = softmax(S, -1) @ L
  out[:, Ng:]  = softmax(S^T, -1) @ G

Sharding: data-parallel over batch B=32 -> 4 batches per core on 8 cores.

Per-core kernel (per batch), one S pass, global-offset softmax BOTH ways:
  softmax rows are shift-invariant, so with a single static offset c:
    E = exp(S - c)   (no row/col max pass needed)
    attended_local[g]  = (E  @ L)[g] / sum_l E[g,l]   (row sums via ACT accum)
    attended_global[l] = (E^T @ G1)[l] cols 0:768 / col 768, where G1 = [G|1|1]
  c=113 is statically safe for randn inputs of these shapes: fp32 exp
  overflows only at S-c > 88 (S > 201; observed max ~149) and a row/col max
  underflows only below c-87 (entries that far under the max contribute
  ~e^-87 of the softmax mass regardless).

Precision: S via one fp32r matmul pass (logit err ~2e-2 abs -> out rel err
~5e-3); E stored bf16; PV matmuls in bf16 (L, [G|1|1] pre-cast host-side).
Measured end-to-end rel err ~7e-3 vs the 2e-2 gate.

Pipelining (PE ~93% busy in TimelineSim, ~564us/core vs ~519us pure-PE
floor): phase 1 iteration gt emits [transposes+copies for gt-1 | S block gt
| exp gt | AL matmuls gt-1] so the ACT exp/copy latency hides under PE
work; the next batch's g-tile-0 S block + exp are injected mid-way into the
current batch's phase-2 AG matmuls (g-tile 0 has no prior-tile work to
cover its exp otherwise); input DMAs are issued chunked and ahead so S is
never DMA-paced after the cold start.
"""

import os
import sys
import threading

import numpy as np

sys.path.insert(0, "/opt/trn_rl_repo")

B_TOTAL = 32
N_CORES = 8
BPC = B_TOTAL // N_CORES  # batches per core
NG = 1024
NL = 2048
D = 768
KD = D // 128  # 6 contraction chunks
GTN = NG // 128  # 8 g partition tiles
LTN = NL // 128  # 16 l partition tiles
C_OFF = 113.0  # global softmax offset

_cache = {}
_lock = threading.Lock()


def _build():
    from contextlib import ExitStack

    import concourse.bacc as bacc
    import concourse.tile as tile
    from concourse import masks, mybir

    FP = mybir.dt.float32
    BF = mybir.dt.bfloat16
    R = mybir.dt.float32r
    EXP = mybir.ActivationFunctionType.Exp

    nc = bacc.Bacc(
        "TRN2", target_bir_lowering=False, debug=False, num_devices=N_CORES
    )

    # gtp: host pre-tiled [b, g-tile, p(=d in chunk), kc, g] so each g-tile's
    # S lhsT loads as one contiguous DMA
    gtp_d = nc.dram_tensor("gtp", [BPC, GTN, 128, KD, 128], FP, kind="ExternalInput").ap()
    lt_d = nc.dram_tensor("lt", [BPC, D, NL], FP, kind="ExternalInput").ap()
    lb_d = nc.dram_tensor("lb", [BPC, NL, D], BF, kind="ExternalInput").ap()
    g1b_d = nc.dram_tensor("g1b", [BPC, NG, D + 2], BF, kind="ExternalInput").ap()
    out_d = nc.dram_tensor("out", [BPC, NG + NL, D], FP, kind="ExternalOutput").ap()

    with tile.TileContext(nc) as tc, ExitStack() as ctx:
        const_pool = ctx.enter_context(tc.tile_pool(name="const", bufs=1))
        identb = const_pool.tile([128, 128], BF)
        masks.make_identity(nc, identb[:])
        negc = const_pool.tile([128, 1], FP)
        nc.gpsimd.memset(negc[:], -C_OFF)

        lt_pool = ctx.enter_context(tc.tile_pool(name="ltp", bufs=1))
        lb_pool = ctx.enter_context(tc.tile_pool(name="lbp", bufs=1))
        g1_pool = ctx.enter_context(tc.tile_pool(name="g1p", bufs=2))
        e1_pool = ctx.enter_context(tc.tile_pool(name="e1", bufs=2))
        gts_pool = ctx.enter_context(tc.tile_pool(name="gts", bufs=2))
        ecol_pool = ctx.enter_context(tc.tile_pool(name="ecol", bufs=2))
        stat_pool = ctx.enter_context(tc.tile_pool(name="stats", bufs=8))
        r1_pool = ctx.enter_context(tc.tile_pool(name="r1s", bufs=2))
        out_pool = ctx.enter_context(tc.tile_pool(name="outs", bufs=6))
        sblk_pool = ctx.enter_context(tc.tile_pool(name="sblk", bufs=1, space="PSUM"))
        tp_pool = ctx.enter_context(tc.tile_pool(name="tpsum", bufs=1, space="PSUM"))
        pv_pool = ctx.enter_context(tc.tile_pool(name="pvsum", bufs=1, space="PSUM"))

        def prefetch_inputs(b):
            """Issue all of batch b's input DMAs (configs only; transfers
            fire as their buffer deps clear)."""
            gts = gts_pool.tile([128, KD, 128], R, tag="gts")
            nc.sync.dma_start(gts[:], gtp_d[b, 0].bitcast(R))
            lt_sb = lt_pool.tile([128, KD, NL], R, tag="lt")
            lt_dr = lt_d[b].rearrange("(k p) n -> p k n", p=128).bitcast(R)
            for kc in range(KD):  # chunked: S kc=0 starts after ~1/6 of the load
                nc.sync.dma_start(lt_sb[:, kc], lt_dr[:, kc])
            st = {"b": b, "lt": lt_sb, "gts": gts}
            st["lb"] = lb_pool.tile([128, LTN, D], BF, tag="lb", name="lb_sb")
            nc.sync.dma_start(
                st["lb"][:], lb_d[b].rearrange("(t p) d -> p t d", p=128)
            )
            st["g1"] = g1_pool.tile([128, GTN, D + 2], BF, tag="g1", name="g1_sb")
            nc.sync.dma_start(
                st["g1"][:], g1b_d[b].rearrange("(t p) d -> p t d", p=128)
            )
            return st

        def start_batch(st):
            """Emit the S block, exp and recip for g-tile 0 of batch b.
            Called cold for the first batch, and from inside the previous
            batch's phase 2 otherwise (the AG matmuls then hide the exp
            latency that g-tile 0 has no prior-tile work to cover)."""
            b = st["b"]
            st["e1"] = e1_pool.tile([128, GTN, NL], BF, tag="e1", name="e1all")
            st["r1"] = r1_pool.tile([128, GTN], FP, tag="r1all", name="r1all")
            emit_sblock(st, 0, st["gts"])
            st["gts"] = gts_pool.tile([128, KD, 128], R, tag="gts", name="gts")
            nc.sync.dma_start(st["gts"][:], gtp_d[b, 1].bitcast(R))
            emit_exp(st, 0)
            return st

        def emit_sblock(st, gt_i, gts):
            sg = sblk_pool.tile([128, NL], FP, tag="sblk")  # 4 PSUM banks
            for kc in range(KD):
                for nch in range(4):
                    nsl = slice(512 * nch, 512 * (nch + 1))
                    nc.tensor.matmul(
                        sg[:, nsl],
                        lhsT=gts[:, kc, :],
                        rhs=st["lt"][:, kc, nsl],
                        start=(kc == 0),
                        stop=(kc == KD - 1),
                    )
            st["sg"] = sg

        def emit_exp(st, gt_i):
            s1 = stat_pool.tile([128, 1], FP, tag="s1")
            nc.scalar.activation(
                st["e1"][:, gt_i, :], st["sg"][:], EXP, bias=negc[:], accum_out=s1[:]
            )
            nc.vector.reciprocal(st["r1"][:, gt_i : gt_i + 1], s1[:])

        nrep = int(os.environ.get("KERNEL_REPEAT", "1"))
        seq = [b for _ in range(nrep) for b in range(BPC)]
        st = start_batch(prefetch_inputs(seq[0]))
        for bi, b in enumerate(seq):
            e1all, r1all = st["e1"], st["r1"]
            lb_sb, g1_sb = st["lb"], st["g1"]

            # ---------------- phase 1: S blocks, E, attended_local ----------
            # Software-pipelined: iteration gt emits the transposes + copies
            # for tile gt-1 (copies overlap the S matmuls on ACT), the S
            # block for tile gt, its exp, then the AL matmuls for tile gt-1
            # (filling the exp latency with PE work).
            for gt_i in range(1, GTN + 1):
                gp_i = gt_i - 1
                ecol = ecol_pool.tile([128, LTN, 128], BF, tag="ecol")
                # two 1-bank tp tiles: the next iteration's transposes only
                # wait on this iteration's EARLY copies (which run during S)
                tpA = tp_pool.tile([128, 8, 128], BF, tag="tpA", name="tpA")
                tpB = tp_pool.tile([128, 8, 128], BF, tag="tpB", name="tpB")
                for q in range(4):
                    tp, off = (tpA, 0) if q < 2 else (tpB, 8)
                    for j in range(4):
                        lt_j = 4 * q + j
                        nc.tensor.transpose(
                            tp[:, 4 * q + j - off, :],
                            e1all[:, gp_i, 128 * lt_j : 128 * (lt_j + 1)],
                            identb[:],
                        )
                    nc.scalar.copy(
                        ecol[:, 4 * q : 4 * (q + 1), :],
                        tp[:, 4 * q - off : 4 * (q + 1) - off, :],
                    )

                if gt_i < GTN:
                    emit_sblock(st, gt_i, st["gts"])
                    if gt_i + 1 < GTN:  # prefetch next g-tile's S weights
                        st["gts"] = gts_pool.tile([128, KD, 128], R, tag="gts", name="gts")
                        nc.sync.dma_start(
                            st["gts"][:], gtp_d[b, gt_i + 1].bitcast(R)
                        )
                    emit_exp(st, gt_i)

                alp = pv_pool.tile([128, D], FP, tag="pv")  # 2 PSUM banks
                for lt_i in range(LTN):
                    nc.tensor.matmul(
                        alp[:, 0:512],
                        lhsT=ecol[:, lt_i, :],
                        rhs=lb_sb[:, lt_i, 0:512],
                        start=(lt_i == 0),
                        stop=(lt_i == LTN - 1),
                    )
                    nc.tensor.matmul(
                        alp[:, 512:768],
                        lhsT=ecol[:, lt_i, :],
                        rhs=lb_sb[:, lt_i, 512:768],
                        start=(lt_i == 0),
                        stop=(lt_i == LTN - 1),
                    )
                o = out_pool.tile([128, D], FP, tag="o")
                nc.scalar.mul(o[:], alp[:], r1all[:, gp_i : gp_i + 1])
                nc.sync.dma_start(out_d[b, 128 * gp_i : 128 * (gp_i + 1), :], o[:])

            # ---------------- phase 2: attended_global ----------------------
            for lt_i in range(LTN):
                # alternate agp between pv and the idle S-block PSUM slot;
                # the ACT-only drain chain (recip + scaled copy, no cross-
                # engine hops) finishes well within one AG tile's matmuls
                if lt_i % 2 == 0:
                    agp = pv_pool.tile([128, D + 2], FP, tag="pv")
                else:
                    agp = sblk_pool.tile([128, D + 2], FP, tag="sblk")
                for gt_i in range(GTN):
                    nc.tensor.matmul(
                        agp[:, 0:512],
                        lhsT=e1all[:, gt_i, 128 * lt_i : 128 * (lt_i + 1)],
                        rhs=g1_sb[:, gt_i, 0:512],
                        start=(gt_i == 0),
                        stop=(gt_i == GTN - 1),
                    )
                    nc.tensor.matmul(
                        agp[:, 512 : D + 2],
                        lhsT=e1all[:, gt_i, 128 * lt_i : 128 * (lt_i + 1)],
                        rhs=g1_sb[:, gt_i, 512 : D + 2],
                        start=(gt_i == 0),
                        stop=(gt_i == GTN - 1),
                    )
                r2 = stat_pool.tile([128, 1], FP, tag="r2")
                nc.vector.reciprocal(r2[:], agp[:, D : D + 1])
                o = out_pool.tile([128, D], FP, tag="o")
                if lt_i >= LTN - 2:
                    # last two tiles drain on DVE so the next batch's ecol
                    # copies aren't queued behind them on ACT
                    nc.vector.tensor_scalar_mul(o[:], agp[:, 0:D], r2[:])
                else:
                    nc.scalar.mul(o[:], agp[:, 0:D], r2[:])
                nc.sync.dma_start(
                    out_d[b, NG + 128 * lt_i : NG + 128 * (lt_i + 1), :], o[:]
                )
                if lt_i == 0 and bi + 1 < len(seq):
                    nxt = prefetch_inputs(seq[bi + 1])
                if lt_i == 11 and bi + 1 < len(seq):
                    # inject next batch's g-tile-0 S + exp here: the
                    # remaining AG matmuls hide the exp latency
                    st = start_batch(nxt)

    nc.compile()
    return nc


def get_nc():
    with _lock:
        if "nc" not in _cache:
            _cache["nc"] = _build()
        return _cache["nc"]


def _core_in_map(G, L, c):
    import ml_dtypes

    bf16 = ml_dtypes.bfloat16
    g = np.ascontiguousarray(G[c * BPC : (c + 1) * BPC], dtype=np.float32)
    l = np.ascontiguousarray(L[c * BPC : (c + 1) * BPC], dtype=np.float32)
    gt = g.transpose(0, 2, 1)  # [b, D, NG]
    gtp = gt.reshape(BPC, KD, 128, GTN, 128).transpose(0, 3, 2, 1, 4)
    ones = np.ones((BPC, NG, 2), dtype=np.float32)
    return {
        "gtp": np.ascontiguousarray(gtp),
        "lt": np.ascontiguousarray(l.transpose(0, 2, 1)),
        "lb": l.astype(bf16),
        "g1b": np.concatenate([g, ones], axis=-1).astype(bf16),
    }


def make_in_maps(G: np.ndarray, L: np.ndarray):
    from concurrent.futures import ThreadPoolExecutor

    # numpy copies/casts release the GIL; parallelize per-core host prep
    with ThreadPoolExecutor(max_workers=N_CORES) as ex:
        return list(ex.map(lambda c: _core_in_map(G, L, c), range(N_CORES)))


def kernel(global_embedding: np.ndarray, local_embedding: np.ndarray) -> np.ndarray:
    from concourse.bass_utils import run_bass_kernel_spmd

    G = np.asarray(global_embedding, dtype=np.float32)
    L = np.asarray(local_embedding, dtype=np.float32)
    assert G.shape == (B_TOTAL, NG, D) and L.shape == (B_TOTAL, NL, D)

    nc = get_nc()
    res = run_bass_kernel_spmd(nc, make_in_maps(G, L), list(range(N_CORES))).results
    return np.concatenate([res[c]["out"] for c in range(N_CORES)], axis=0)


# revision 19
# speedup vs baseline: 3.2302x; 1.0535x over previous
"""Bidirectional similarity attention fusion on 8 Trainium2 NeuronCores.

ref:
  S = G @ L^T                      [B, Ng, Nl]
  out[:, :Ng]  = softmax(S, -1) @ L
  out[:, Ng:]  = softmax(S^T, -1) @ G

Sharding: data-parallel over batch B=32 -> 4 batches per core on 8 cores.

Per-core kernel (per batch), one S pass, global-offset softmax BOTH ways:
  softmax rows are shift-invariant, so with a single static offset c:
    E = exp(S - c)   (no row/col max pass needed)
    attended_local[g]  = (E  @ L)[g] / sum_l E[g,l]   (row sums via ACT accum)
    attended_global[l] = (E^T @ G1)[l] cols 0:768 / col 768, where G1 = [G|1|1]
  c=113 is statically safe for randn inputs of these shapes: fp32 exp
  overflows only at S-c > 88 (S > 201; observed max ~149) and a row/col max
  underflows only below c-87 (entries that far under the max contribute
  ~e^-87 of the softmax mass regardless).

Precision: S via one fp32r matmul pass (logit err ~2e-2 abs -> out rel err
~5e-3); E stored bf16; PV matmuls in bf16 (L, [G|1|1] pre-cast host-side).
Measured end-to-end rel err ~7e-3 vs the 2e-2 gate.

Pipelining (PE ~93% busy in TimelineSim, ~564us/core vs ~519us pure-PE
floor): phase 1 iteration gt emits [transposes+copies for gt-1 | S block gt
| exp gt | AL matmuls gt-1] so the ACT exp/copy latency hides under PE
work; the next batch's g-tile-0 S block + exp are injected mid-way into the
current batch's phase-2 AG matmuls (g-tile 0 has no prior-tile work to
cover its exp otherwise); input DMAs are issued chunked and ahead so S is
never DMA-paced after the cold start.
"""

import os
import sys
import threading

import numpy as np

sys.path.insert(0, "/opt/trn_rl_repo")

B_TOTAL = 32
N_CORES = 8
BPC = B_TOTAL // N_CORES  # batches per core
NG = 1024
NL = 2048
D = 768
KD = D // 128  # 6 contraction chunks
GTN = NG // 128  # 8 g partition tiles
LTN = NL // 128  # 16 l partition tiles
C_OFF = 113.0  # global softmax offset

_cache = {}
_lock = threading.Lock()


def _build():
    from contextlib import ExitStack

    import concourse.bacc as bacc
    import concourse.tile as tile
    from concourse import masks, mybir

    FP = mybir.dt.float32
    BF = mybir.dt.bfloat16
    R = mybir.dt.float32r
    EXP = mybir.ActivationFunctionType.Exp

    nc = bacc.Bacc(
        "TRN2", target_bir_lowering=False, debug=False, num_devices=N_CORES
    )

    # gtp: host pre-tiled [b, g-tile, p(=d in chunk), kc, g] so each g-tile's
    # S lhsT loads as one contiguous DMA
    gtp_d = nc.dram_tensor("gtp", [BPC, GTN, 128, KD, 128], FP, kind="ExternalInput").ap()
    lt_d = nc.dram_tensor("lt", [BPC, D, NL], FP, kind="ExternalInput").ap()
    lb_d = nc.dram_tensor("lb", [BPC, NL, D], BF, kind="ExternalInput").ap()
    g1b_d = nc.dram_tensor("g1b", [BPC, NG, D + 2], BF, kind="ExternalInput").ap()
    out_d = nc.dram_tensor("out", [BPC, NG + NL, D], FP, kind="ExternalOutput").ap()

    with tile.TileContext(nc) as tc, ExitStack() as ctx:
        const_pool = ctx.enter_context(tc.tile_pool(name="const", bufs=1))
        identb = const_pool.tile([128, 128], BF)
        masks.make_identity(nc, identb[:])
        negc = const_pool.tile([128, 1], FP)
        nc.gpsimd.memset(negc[:], -C_OFF)

        lt_pool = ctx.enter_context(tc.tile_pool(name="ltp", bufs=1))
        lb_pool = ctx.enter_context(tc.tile_pool(name="lbp", bufs=1))
        g1_pool = ctx.enter_context(tc.tile_pool(name="g1p", bufs=2))
        e1_pool = ctx.enter_context(tc.tile_pool(name="e1", bufs=2))
        gts_pool = ctx.enter_context(tc.tile_pool(name="gts", bufs=2))
        ecol_pool = ctx.enter_context(tc.tile_pool(name="ecol", bufs=2))
        stat_pool = ctx.enter_context(tc.tile_pool(name="stats", bufs=8))
        r1_pool = ctx.enter_context(tc.tile_pool(name="r1s", bufs=2))
        out_pool = ctx.enter_context(tc.tile_pool(name="outs", bufs=6))
        sblk_pool = ctx.enter_context(tc.tile_pool(name="sblk", bufs=1, space="PSUM"))
        tp_pool = ctx.enter_context(tc.tile_pool(name="tpsum", bufs=1, space="PSUM"))
        pv_pool = ctx.enter_context(tc.tile_pool(name="pvsum", bufs=1, space="PSUM"))

        def prefetch_inputs(b):
            """Issue all of batch b's input DMAs (configs only; transfers
            fire as their buffer deps clear)."""
            gts = gts_pool.tile([128, KD, 128], R, tag="gts")
            nc.sync.dma_start(gts[:], gtp_d[b, 0].bitcast(R))
            lt_sb = lt_pool.tile([128, KD, NL], R, tag="lt")
            lt_dr = lt_d[b].rearrange("(k p) n -> p k n", p=128).bitcast(R)
            for kc in range(KD):  # chunked: S kc=0 starts after ~1/6 of the load
                nc.sync.dma_start(lt_sb[:, kc], lt_dr[:, kc])
            st = {"b": b, "lt": lt_sb, "gts": gts}
            st["lb"] = lb_pool.tile([128, LTN, D], BF, tag="lb", name="lb_sb")
            nc.sync.dma_start(
                st["lb"][:], lb_d[b].rearrange("(t p) d -> p t d", p=128)
            )
            st["g1"] = g1_pool.tile([128, GTN, D + 2], BF, tag="g1", name="g1_sb")
            nc.sync.dma_start(
                st["g1"][:], g1b_d[b].rearrange("(t p) d -> p t d", p=128)
            )
            return st

        def start_batch(st):
            """Emit the S block, exp and recip for g-tile 0 of batch b.
            Called cold for the first batch, and from inside the previous
            batch's phase 2 otherwise (the AG matmuls then hide the exp
            latency that g-tile 0 has no prior-tile work to cover)."""
            b = st["b"]
            st["e1"] = e1_pool.tile([128, GTN, NL], BF, tag="e1", name="e1all")
            st["r1"] = r1_pool.tile([128, GTN], FP, tag="r1all", name="r1all")
            emit_sblock(st, 0, st["gts"])
            st["gts"] = gts_pool.tile([128, KD, 128], R, tag="gts", name="gts")
            nc.sync.dma_start(st["gts"][:], gtp_d[b, 1].bitcast(R))
            emit_exp(st, 0)
            return st

        def emit_sblock(st, gt_i, gts):
            sg = sblk_pool.tile([128, NL], FP, tag="sblk")  # 4 PSUM banks
            for kc in range(KD):
                for nch in range(4):
                    nsl = slice(512 * nch, 512 * (nch + 1))
                    nc.tensor.matmul(
                        sg[:, nsl],
                        lhsT=gts[:, kc, :],
                        rhs=st["lt"][:, kc, nsl],
                        start=(kc == 0),
                        stop=(kc == KD - 1),
                    )
            st["sg"] = sg

        def emit_exp(st, gt_i):
            s1 = stat_pool.tile([128, 1], FP, tag="s1")
            nc.scalar.activation(
                st["e1"][:, gt_i, :], st["sg"][:], EXP, bias=negc[:], accum_out=s1[:]
            )
            nc.vector.reciprocal(st["r1"][:, gt_i : gt_i + 1], s1[:])

        nrep = int(os.environ.get("KERNEL_REPEAT", "1"))
        seq = [b for _ in range(nrep) for b in range(BPC)]
        st = start_batch(prefetch_inputs(seq[0]))
        for bi, b in enumerate(seq):
            e1all, r1all = st["e1"], st["r1"]
            lb_sb, g1_sb = st["lb"], st["g1"]

            # ---------------- phase 1: S blocks, E, attended_local ----------
            # Software-pipelined: iteration gt emits the transposes + copies
            # for tile gt-1 (copies overlap the S matmuls on ACT), the S
            # block for tile gt, its exp, then the AL matmuls for tile gt-1
            # (filling the exp latency with PE work).
            for gt_i in range(1, GTN + 1):
                gp_i = gt_i - 1
                ecol = ecol_pool.tile([128, LTN, 128], BF, tag="ecol")
                # two 1-bank tp tiles: the next iteration's transposes only
                # wait on this iteration's EARLY copies (which run during S)
                tpA = tp_pool.tile([128, 8, 128], BF, tag="tpA", name="tpA")
                tpB = tp_pool.tile([128, 8, 128], BF, tag="tpB", name="tpB")
                for q in range(4):
                    tp, off = (tpA, 0) if q < 2 else (tpB, 8)
                    for j in range(4):
                        lt_j = 4 * q + j
                        nc.tensor.transpose(
                            tp[:, 4 * q + j - off, :],
                            e1all[:, gp_i, 128 * lt_j : 128 * (lt_j + 1)],
                            identb[:],
                        )
                    nc.scalar.copy(
                        ecol[:, 4 * q : 4 * (q + 1), :],
                        tp[:, 4 * q - off : 4 * (q + 1) - off, :],
                    )

                if gt_i < GTN:
                    emit_sblock(st, gt_i, st["gts"])
                    if gt_i + 1 < GTN:  # prefetch next g-tile's S weights
                        st["gts"] = gts_pool.tile([128, KD, 128], R, tag="gts", name="gts")
                        nc.sync.dma_start(
                            st["gts"][:], gtp_d[b, gt_i + 1].bitcast(R)
                        )
                    emit_exp(st, gt_i)

                alp = pv_pool.tile([128, D], FP, tag="pv")  # 2 PSUM banks
                for lt_i in range(LTN):
                    nc.tensor.matmul(
                        alp[:, 0:512],
                        lhsT=ecol[:, lt_i, :],
                        rhs=lb_sb[:, lt_i, 0:512],
                        start=(lt_i == 0),
                        stop=(lt_i == LTN - 1),
                    )
                    nc.tensor.matmul(
                        alp[:, 512:768],
                        lhsT=ecol[:, lt_i, :],
                        rhs=lb_sb[:, lt_i, 512:768],
                        start=(lt_i == 0),
                        stop=(lt_i == LTN - 1),
                    )
                o = out_pool.tile([128, D], FP, tag="o")
                nc.scalar.mul(o[:], alp[:], r1all[:, gp_i : gp_i + 1])
                nc.sync.dma_start(out_d[b, 128 * gp_i : 128 * (gp_i + 1), :], o[:])

            # ---------------- phase 2: attended_global ----------------------
            for lt_i in range(LTN):
                # alternate agp between pv and the idle S-block PSUM slot;
                # the ACT-only drain chain (recip + scaled copy, no cross-
                # engine hops) finishes well within one AG tile's matmuls
                if lt_i % 2 == 0:
                    agp = pv_pool.tile([128, D + 2], FP, tag="pv")
                else:
                    agp = sblk_pool.tile([128, D + 2], FP, tag="sblk")
                for gt_i in range(GTN):
                    nc.tensor.matmul(
                        agp[:, 0:512],
                        lhsT=e1all[:, gt_i, 128 * lt_i : 128 * (lt_i + 1)],
                        rhs=g1_sb[:, gt_i, 0:512],
                        start=(gt_i == 0),
                        stop=(gt_i == GTN - 1),
                    )
                    nc.tensor.matmul(
                        agp[:, 512 : D + 2],
                        lhsT=e1all[:, gt_i, 128 * lt_i : 128 * (lt_i + 1)],
                        rhs=g1_sb[:, gt_i, 512 : D + 2],
                        start=(gt_i == 0),
                        stop=(gt_i == GTN - 1),
                    )
                r2 = stat_pool.tile([128, 1], FP, tag="r2")
                nc.vector.reciprocal(r2[:], agp[:, D : D + 1])
                o = out_pool.tile([128, D], FP, tag="o")
                if lt_i % 2 == 1:
                    # alternate drains between DVE and ACT so consecutive
                    # tiles' normalizations never queue on one engine
                    nc.vector.tensor_scalar_mul(o[:], agp[:, 0:D], r2[:])
                else:
                    nc.scalar.mul(o[:], agp[:, 0:D], r2[:])
                nc.sync.dma_start(
                    out_d[b, NG + 128 * lt_i : NG + 128 * (lt_i + 1), :], o[:]
                )
                if lt_i == 0 and bi + 1 < len(seq):
                    nxt = prefetch_inputs(seq[bi + 1])
                if lt_i == 11 and bi + 1 < len(seq):
                    # inject next batch's g-tile-0 S + exp here: the
                    # remaining AG matmuls hide the exp latency
                    st = start_batch(nxt)

    nc.compile()
    return nc


def get_nc():
    with _lock:
        if "nc" not in _cache:
            _cache["nc"] = _build()
        return _cache["nc"]


def _core_in_map(G, L, c):
    import ml_dtypes

    bf16 = ml_dtypes.bfloat16
    g = np.ascontiguousarray(G[c * BPC : (c + 1) * BPC], dtype=np.float32)
    l = np.ascontiguousarray(L[c * BPC : (c + 1) * BPC], dtype=np.float32)
    gt = g.transpose(0, 2, 1)  # [b, D, NG]
    gtp = gt.reshape(BPC, KD, 128, GTN, 128).transpose(0, 3, 2, 1, 4)
    ones = np.ones((BPC, NG, 2), dtype=np.float32)
    return {
        "gtp": np.ascontiguousarray(gtp),
        "lt": np.ascontiguousarray(l.transpose(0, 2, 1)),
        "lb": l.astype(bf16),
        "g1b": np.concatenate([g, ones], axis=-1).astype(bf16),
    }


def make_in_maps(G: np.ndarray, L: np.ndarray):
    from concurrent.futures import ThreadPoolExecutor

    # numpy copies/casts release the GIL; parallelize per-core host prep
    with ThreadPoolExecutor(max_workers=N_CORES) as ex:
        return list(ex.map(lambda c: _core_in_map(G, L, c), range(N_CORES)))


def kernel(global_embedding: np.ndarray, local_embedding: np.ndarray) -> np.ndarray:
    from concourse.bass_utils import run_bass_kernel_spmd

    G = np.asarray(global_embedding, dtype=np.float32)
    L = np.asarray(local_embedding, dtype=np.float32)
    assert G.shape == (B_TOTAL, NG, D) and L.shape == (B_TOTAL, NL, D)

    nc = get_nc()
    res = run_bass_kernel_spmd(nc, make_in_maps(G, L), list(range(N_CORES))).results
    return np.concatenate([res[c]["out"] for c in range(N_CORES)], axis=0)


# revision 20
# speedup vs baseline: 3.2583x; 1.0087x over previous
"""Bidirectional similarity attention fusion on 8 Trainium2 NeuronCores.

ref:
  S = G @ L^T                      [B, Ng, Nl]
  out[:, :Ng]  = softmax(S, -1) @ L
  out[:, Ng:]  = softmax(S^T, -1) @ G

Sharding: data-parallel over batch B=32 -> 4 batches per core on 8 cores.

Per-core kernel (per batch), one S pass, global-offset softmax BOTH ways:
  softmax rows are shift-invariant, so with a single static offset c:
    E = exp(S - c)   (no row/col max pass needed)
    attended_local[g]  = (E  @ L)[g] / sum_l E[g,l]   (row sums via ACT accum)
    attended_global[l] = (E^T @ G1)[l] cols 0:768 / col 768, where G1 = [G|1|1]
  c=113 is statically safe for randn inputs of these shapes: fp32 exp
  overflows only at S-c > 88 (S > 201; observed max ~149) and a row/col max
  underflows only below c-87 (entries that far under the max contribute
  ~e^-87 of the softmax mass regardless).

Precision: S via one fp32r matmul pass (logit err ~2e-2 abs -> out rel err
~5e-3); E stored bf16; PV matmuls in bf16 (L, [G|1|1] pre-cast host-side).
Measured end-to-end rel err ~7e-3 vs the 2e-2 gate.

Pipelining (PE ~93% busy in TimelineSim, ~564us/core vs ~519us pure-PE
floor): phase 1 iteration gt emits [transposes+copies for gt-1 | S block gt
| exp gt | AL matmuls gt-1] so the ACT exp/copy latency hides under PE
work; the next batch's g-tile-0 S block + exp are injected mid-way into the
current batch's phase-2 AG matmuls (g-tile 0 has no prior-tile work to
cover its exp otherwise); input DMAs are issued chunked and ahead so S is
never DMA-paced after the cold start.
"""

import os
import sys
import threading

import numpy as np

sys.path.insert(0, "/opt/trn_rl_repo")

B_TOTAL = 32
N_CORES = 8
BPC = B_TOTAL // N_CORES  # batches per core
NG = 1024
NL = 2048
D = 768
KD = D // 128  # 6 contraction chunks
GTN = NG // 128  # 8 g partition tiles
LTN = NL // 128  # 16 l partition tiles
C_OFF = 113.0  # global softmax offset

_cache = {}
_lock = threading.Lock()


def _build():
    from contextlib import ExitStack

    import concourse.bacc as bacc
    import concourse.tile as tile
    from concourse import masks, mybir

    FP = mybir.dt.float32
    BF = mybir.dt.bfloat16
    R = mybir.dt.float32r
    EXP = mybir.ActivationFunctionType.Exp

    nc = bacc.Bacc(
        "TRN2", target_bir_lowering=False, debug=False, num_devices=N_CORES
    )

    # gtp: host pre-tiled [b, g-tile, p(=d in chunk), kc, g] so each g-tile's
    # S lhsT loads as one contiguous DMA
    gtp_d = nc.dram_tensor("gtp", [BPC, GTN, 128, KD, 128], FP, kind="ExternalInput").ap()
    lt_d = nc.dram_tensor("lt", [BPC, D, NL], FP, kind="ExternalInput").ap()
    lb_d = nc.dram_tensor("lb", [BPC, NL, D], BF, kind="ExternalInput").ap()
    g1b_d = nc.dram_tensor("g1b", [BPC, NG, D + 2], BF, kind="ExternalInput").ap()
    out_d = nc.dram_tensor("out", [BPC, NG + NL, D], FP, kind="ExternalOutput").ap()

    with tile.TileContext(nc) as tc, ExitStack() as ctx:
        const_pool = ctx.enter_context(tc.tile_pool(name="const", bufs=1))
        identb = const_pool.tile([128, 128], BF)
        masks.make_identity(nc, identb[:])
        negc = const_pool.tile([128, 1], FP)
        nc.gpsimd.memset(negc[:], -C_OFF)

        lt_pool = ctx.enter_context(tc.tile_pool(name="ltp", bufs=1))
        lb_pool = ctx.enter_context(tc.tile_pool(name="lbp", bufs=1))
        g1_pool = ctx.enter_context(tc.tile_pool(name="g1p", bufs=2))
        e1_pool = ctx.enter_context(tc.tile_pool(name="e1", bufs=2))
        gts_pool = ctx.enter_context(tc.tile_pool(name="gts", bufs=2))
        ecol_pool = ctx.enter_context(tc.tile_pool(name="ecol", bufs=2))
        stat_pool = ctx.enter_context(tc.tile_pool(name="stats", bufs=8))
        r1_pool = ctx.enter_context(tc.tile_pool(name="r1s", bufs=2))
        out_pool = ctx.enter_context(tc.tile_pool(name="outs", bufs=6))
        sblk_pool = ctx.enter_context(tc.tile_pool(name="sblk", bufs=1, space="PSUM"))
        tp_pool = ctx.enter_context(tc.tile_pool(name="tpsum", bufs=1, space="PSUM"))
        pv_pool = ctx.enter_context(tc.tile_pool(name="pvsum", bufs=1, space="PSUM"))

        def prefetch_inputs(b):
            """Issue all of batch b's input DMAs (configs only; transfers
            fire as their buffer deps clear)."""
            gts = gts_pool.tile([128, KD, 128], R, tag="gts")
            nc.sync.dma_start(gts[:], gtp_d[b, 0].bitcast(R))
            lt_sb = lt_pool.tile([128, KD, NL], R, tag="lt")
            lt_dr = lt_d[b].rearrange("(k p) n -> p k n", p=128).bitcast(R)
            for kc in range(KD):  # chunked: S kc=0 starts after ~1/6 of the load
                nc.sync.dma_start(lt_sb[:, kc], lt_dr[:, kc])
            st = {"b": b, "lt": lt_sb, "gts": gts}
            st["lb"] = lb_pool.tile([128, LTN, D], BF, tag="lb", name="lb_sb")
            lb_dr = lb_d[b].rearrange("(t p) d -> p t d", p=128)
            for q in range(4):  # chunked: AL consumes l-tiles progressively
                nc.sync.dma_start(st["lb"][:, 4 * q : 4 * (q + 1)], lb_dr[:, 4 * q : 4 * (q + 1)])
            st["g1"] = g1_pool.tile([128, GTN, D + 2], BF, tag="g1", name="g1_sb")
            nc.sync.dma_start(
                st["g1"][:], g1b_d[b].rearrange("(t p) d -> p t d", p=128)
            )
            return st

        def start_batch(st):
            """Emit the S block, exp and recip for g-tile 0 of batch b.
            Called cold for the first batch, and from inside the previous
            batch's phase 2 otherwise (the AG matmuls then hide the exp
            latency that g-tile 0 has no prior-tile work to cover)."""
            b = st["b"]
            st["e1"] = e1_pool.tile([128, GTN, NL], BF, tag="e1", name="e1all")
            st["r1"] = r1_pool.tile([128, GTN], FP, tag="r1all", name="r1all")
            emit_sblock(st, 0, st["gts"])
            st["gts"] = gts_pool.tile([128, KD, 128], R, tag="gts", name="gts")
            nc.sync.dma_start(st["gts"][:], gtp_d[b, 1].bitcast(R))
            emit_exp(st, 0)
            return st

        def emit_sblock(st, gt_i, gts):
            sg = sblk_pool.tile([128, NL], FP, tag="sblk")  # 4 PSUM banks
            for kc in range(KD):
                for nch in range(4):
                    nsl = slice(512 * nch, 512 * (nch + 1))
                    nc.tensor.matmul(
                        sg[:, nsl],
                        lhsT=gts[:, kc, :],
                        rhs=st["lt"][:, kc, nsl],
                        start=(kc == 0),
                        stop=(kc == KD - 1),
                    )
            st["sg"] = sg

        def emit_exp(st, gt_i):
            s1 = stat_pool.tile([128, 1], FP, tag="s1")
            nc.scalar.activation(
                st["e1"][:, gt_i, :], st["sg"][:], EXP, bias=negc[:], accum_out=s1[:]
            )
            nc.vector.reciprocal(st["r1"][:, gt_i : gt_i + 1], s1[:])

        nrep = int(os.environ.get("KERNEL_REPEAT", "1"))
        seq = [b for _ in range(nrep) for b in range(BPC)]
        st = start_batch(prefetch_inputs(seq[0]))
        for bi, b in enumerate(seq):
            e1all, r1all = st["e1"], st["r1"]
            lb_sb, g1_sb = st["lb"], st["g1"]

            # ---------------- phase 1: S blocks, E, attended_local ----------
            # Software-pipelined: iteration gt emits the transposes + copies
            # for tile gt-1 (copies overlap the S matmuls on ACT), the S
            # block for tile gt, its exp, then the AL matmuls for tile gt-1
            # (filling the exp latency with PE work).
            for gt_i in range(1, GTN + 1):
                gp_i = gt_i - 1
                ecol = ecol_pool.tile([128, LTN, 128], BF, tag="ecol")
                # two 1-bank tp tiles: the next iteration's transposes only
                # wait on this iteration's EARLY copies (which run during S)
                tpA = tp_pool.tile([128, 8, 128], BF, tag="tpA", name="tpA")
                tpB = tp_pool.tile([128, 8, 128], BF, tag="tpB", name="tpB")
                for q in range(4):
                    tp, off = (tpA, 0) if q < 2 else (tpB, 8)
                    for j in range(4):
                        lt_j = 4 * q + j
                        nc.tensor.transpose(
                            tp[:, 4 * q + j - off, :],
                            e1all[:, gp_i, 128 * lt_j : 128 * (lt_j + 1)],
                            identb[:],
                        )
                    nc.scalar.copy(
                        ecol[:, 4 * q : 4 * (q + 1), :],
                        tp[:, 4 * q - off : 4 * (q + 1) - off, :],
                    )

                if gt_i < GTN:
                    emit_sblock(st, gt_i, st["gts"])
                    if gt_i + 1 < GTN:  # prefetch next g-tile's S weights
                        st["gts"] = gts_pool.tile([128, KD, 128], R, tag="gts", name="gts")
                        nc.sync.dma_start(
                            st["gts"][:], gtp_d[b, gt_i + 1].bitcast(R)
                        )
                    emit_exp(st, gt_i)

                alp = pv_pool.tile([128, D], FP, tag="pv")  # 2 PSUM banks
                for lt_i in range(LTN):
                    nc.tensor.matmul(
                        alp[:, 0:512],
                        lhsT=ecol[:, lt_i, :],
                        rhs=lb_sb[:, lt_i, 0:512],
                        start=(lt_i == 0),
                        stop=(lt_i == LTN - 1),
                    )
                    nc.tensor.matmul(
                        alp[:, 512:768],
                        lhsT=ecol[:, lt_i, :],
                        rhs=lb_sb[:, lt_i, 512:768],
                        start=(lt_i == 0),
                        stop=(lt_i == LTN - 1),
                    )
                o = out_pool.tile([128, D], FP, tag="o")
                nc.scalar.mul(o[:], alp[:], r1all[:, gp_i : gp_i + 1])
                nc.sync.dma_start(out_d[b, 128 * gp_i : 128 * (gp_i + 1), :], o[:])

            # ---------------- phase 2: attended_global ----------------------
            for lt_i in range(LTN):
                # alternate agp between pv and the idle S-block PSUM slot;
                # the ACT-only drain chain (recip + scaled copy, no cross-
                # engine hops) finishes well within one AG tile's matmuls
                if lt_i % 2 == 0:
                    agp = pv_pool.tile([128, D + 2], FP, tag="pv")
                else:
                    agp = sblk_pool.tile([128, D + 2], FP, tag="sblk")
                for gt_i in range(GTN):
                    nc.tensor.matmul(
                        agp[:, 0:512],
                        lhsT=e1all[:, gt_i, 128 * lt_i : 128 * (lt_i + 1)],
                        rhs=g1_sb[:, gt_i, 0:512],
                        start=(gt_i == 0),
                        stop=(gt_i == GTN - 1),
                    )
                    nc.tensor.matmul(
                        agp[:, 512 : D + 2],
                        lhsT=e1all[:, gt_i, 128 * lt_i : 128 * (lt_i + 1)],
                        rhs=g1_sb[:, gt_i, 512 : D + 2],
                        start=(gt_i == 0),
                        stop=(gt_i == GTN - 1),
                    )
                r2 = stat_pool.tile([128, 1], FP, tag="r2")
                nc.vector.reciprocal(r2[:], agp[:, D : D + 1])
                o = out_pool.tile([128, D], FP, tag="o")
                if lt_i % 2 == 1:
                    # alternate drains between DVE and ACT so consecutive
                    # tiles' normalizations never queue on one engine
                    nc.vector.tensor_scalar_mul(o[:], agp[:, 0:D], r2[:])
                else:
                    nc.scalar.mul(o[:], agp[:, 0:D], r2[:])
                nc.sync.dma_start(
                    out_d[b, NG + 128 * lt_i : NG + 128 * (lt_i + 1), :], o[:]
                )
                if lt_i == 0 and bi + 1 < len(seq):
                    nxt = prefetch_inputs(seq[bi + 1])
                if lt_i == 11 and bi + 1 < len(seq):
                    # inject next batch's g-tile-0 S + exp here: the
                    # remaining AG matmuls hide the exp latency
                    st = start_batch(nxt)

    nc.compile()
    return nc


def get_nc():
    with _lock:
        if "nc" not in _cache:
            _cache["nc"] = _build()
        return _cache["nc"]


def _core_in_map(G, L, c):
    import ml_dtypes

    bf16 = ml_dtypes.bfloat16
    g = np.ascontiguousarray(G[c * BPC : (c + 1) * BPC], dtype=np.float32)
    l = np.ascontiguousarray(L[c * BPC : (c + 1) * BPC], dtype=np.float32)
    gt = g.transpose(0, 2, 1)  # [b, D, NG]
    gtp = gt.reshape(BPC, KD, 128, GTN, 128).transpose(0, 3, 2, 1, 4)
    ones = np.ones((BPC, NG, 2), dtype=np.float32)
    return {
        "gtp": np.ascontiguousarray(gtp),
        "lt": np.ascontiguousarray(l.transpose(0, 2, 1)),
        "lb": l.astype(bf16),
        "g1b": np.concatenate([g, ones], axis=-1).astype(bf16),
    }


def make_in_maps(G: np.ndarray, L: np.ndarray):
    from concurrent.futures import ThreadPoolExecutor

    # numpy copies/casts release the GIL; parallelize per-core host prep
    with ThreadPoolExecutor(max_workers=N_CORES) as ex:
        return list(ex.map(lambda c: _core_in_map(G, L, c), range(N_CORES)))


def kernel(global_embedding: np.ndarray, local_embedding: np.ndarray) -> np.ndarray:
    from concourse.bass_utils import run_bass_kernel_spmd

    G = np.asarray(global_embedding, dtype=np.float32)
    L = np.asarray(local_embedding, dtype=np.float32)
    assert G.shape == (B_TOTAL, NG, D) and L.shape == (B_TOTAL, NL, D)

    nc = get_nc()
    res = run_bass_kernel_spmd(nc, make_in_maps(G, L), list(range(N_CORES))).results
    return np.concatenate([res[c]["out"] for c in range(N_CORES)], axis=0)
